# revision 8
# baseline (speedup 1.0000x reference)
"""CPR linear (int8-dequant matmul with column reordering) on 8 Trainium2
NeuronCores.

Math: y = x[:, col_indices] @ (W_int8 * repeat(scales, gs)) + bias
Equivalently, with inv = argsort(col_indices):
    y[m, n] = sum_j x[m, j] * W[inv[j], n] * scales[inv[j]//gs, n]
so x is consumed in natural column order and the permutation rides on W's
rows (host-side index gather; W is 8x smaller than x).

Sharding: column-parallel. Each core owns 512 output features: its slice
of W (row-permuted, dequantized on host) and bias; x is replicated.

Precision/speed scheme along K (32 k-tiles of 128), per-tile class:
  - bf16:  plain bf16 matmul (213ns/tile)
  - corr:  fp8e4m3 with first-order residual correction:
               x8@w8 + dx8@w8 + x8@dw8
           (3 fp8 products = 1.5 DoubleRow instrs; ~bf16 accuracy since the
           residuals kill both operands' quantization noise; only the tiny
           dx*dw cross term is dropped)
  - plain: fp8e4m3 pair (0.5 DoubleRow instr; ~3.75% tile noise)
DoubleRow contracts 2 k-tiles per PE pass at 0.5 cycles/row. The host folds
a per-column power-of-2 scale 2^(A8+cn) into every weight format so all
products accumulate in one PSUM group; eviction rescales by 2^-(A8+cn)
and adds bias. fp8 residuals use the SAME power-of-2 scales so everything
lands at matched magnitude.

Per-core device kernel:
  - weights DMA'd into resident tiles, chunked so the first matmuls gate on
    a ~0.25MB load; bias/colscale broadcast [512] -> [128, 512]
  - loop over 8 m-blocks of 1024 rows:
      x loads on the sync queue ([128, t, 1024] tiles per dtype)
      mb 0: k-outer over all 8 PSUM banks so the PE starts on the first
            x granule instead of waiting for the full block
      mb 1+: m-subtile-outer, bf16/DoubleRow interleaved so DoubleRow
            LDWEIGHTS can hide under long bf16 matmuls
      PSUM * colscale + bias -> SBUF pairs -> stores on the gpsimd queue
"""
from contextlib import ExitStack

import numpy as np
import ml_dtypes

import concourse.bass as bass
import concourse.bacc as bacc
import concourse.mybir as mybir
import concourse.tile as tile

B, S, K, N = 4, 2048, 4096, 4096
M = B * S                    # 8192
NCORES = 8
NS = N // NCORES             # 512 output cols per core
P = 128
NKT = K // P                 # 32 k-tiles
MB = 1024                    # m-block rows
NMB = M // MB                # 8
MSUB = MB // P               # 8

# k-tile classes: [NBF bf16][NCORR fp8-corrected][NPLAIN fp8-plain]
NBF, NCORR, NPLAIN = 0, 26, 6
N8T = NCORR + NPLAIN         # tiles with fp8 x8/w8 operands
assert NBF + N8T == NKT and N8T % 2 == 0 and NCORR % 2 == 0
A8 = 5                       # x fp8 pre-scale: x*2^5 (max |x|*32 < 240)

bf16 = mybir.dt.bfloat16
f32 = mybir.dt.float32
f8e4 = mybir.dt.float8e4

KB = 4                       # k-tiles per steady-state x DMA


def _granules(n_tiles, first_fine, fine):
    """Chunk n_tiles into DMA granule sizes, finest first."""
    out, left = [], n_tiles
    sched = list(first_fine) if first_fine else []
    for g in sched:
        if left <= 0:
            break
        g = min(g, left)
        out.append(g)
        left -= g
    while left > 0:
        g = min(fine, left)
        out.append(g)
        left -= g
    return out


def build(repeats: int = 1, variant: str = "full"):
    """variant: "full" | "nomm" (DMA path only) | "mmonly" (PE path only)"""
    do_mm = variant in ("full", "mmonly")
    do_xdma = variant in ("full", "nomm")

    nc = bacc.Bacc(None)
    # x pre-transposed on host, split by k-tile class
    x_d = x8_d = dx8_d = None
    w_d = w8_d = dw8_d = None
    if NBF:
        x_d = nc.dram_tensor("xbf", [NBF * P, M], bf16, kind="ExternalInput")
        w_d = nc.dram_tensor("wbf", [NBF * P, NS], bf16, kind="ExternalInput")
    x8_d = nc.dram_tensor("x8", [N8T * P, M], f8e4, kind="ExternalInput")
    w8_d = nc.dram_tensor("w8", [N8T * P, NS], f8e4, kind="ExternalInput")
    if NCORR:
        dx8_d = nc.dram_tensor("dx8", [NCORR * P, M], f8e4,
                               kind="ExternalInput")
        dw8_d = nc.dram_tensor("dw8", [NCORR * P, NS], f8e4,
                               kind="ExternalInput")
    b_d = nc.dram_tensor("bias", [NS], f32, kind="ExternalInput")
    cs_d = nc.dram_tensor("colscale", [NS], f32, kind="ExternalInput")
    y_d = nc.dram_tensor("y", [M, NS], f32, kind="ExternalOutput")

    with tile.TileContext(nc) as tc, ExitStack() as stk:
        if repeats > 1:
            stk.enter_context(tc.For_i(0, repeats, 1))
        with (
            tc.tile_pool(name="consts", bufs=1) as consts,
            tc.tile_pool(name="xpool", bufs=2) as xpool,
            tc.tile_pool(name="opool", bufs=2) as opool,
            tc.tile_pool(name="psum", bufs=1, space="PSUM") as psum_pool,
        ):
            # resident weights, chunked smallest-first so early matmuls
            # gate on a small load
            def load_w(dst, src, gran):
                k0 = 0
                for H in gran:
                    nc.scalar.dma_start(
                        out=dst[:, k0:k0 + H],
                        in_=src[k0 * P:(k0 + H) * P, :].rearrange(
                            "(t p) n -> p t n", p=P))
                    k0 += H

            wd = None
            if NBF:
                wd = consts.tile([P, NBF, NS], bf16)
                load_w(wd, w_d, _granules(NBF, (2, 2, 4), 8))
                w8t = consts.tile([P, N8T, NS], f8e4)
                load_w(w8t, w8_d, _granules(N8T, (), 16))
            else:
                w8t = consts.tile([P, N8T, NS], f8e4)
                load_w(w8t, w8_d, _granules(N8T, (2, 2, 4, 8), 16))
            dw8t = None
            if NCORR:
                dw8t = consts.tile([P, NCORR, NS], f8e4)
                load_w(dw8t, dw8_d, _granules(NCORR, (), 16))

            # bias/colscale broadcast to all partitions (needed only at first
            # PSUM eviction, so issued after the W loads on the same queue)
            bias_t = consts.tile([P, NS], f32)
            nc.scalar.dma_start(
                out=bias_t,
                in_=bass.AP(tensor=b_d, offset=0, ap=[[0, P], [1, NS]]),
            )
            cs_t = consts.tile([P, NS], f32)
            nc.scalar.dma_start(
                out=cs_t,
                in_=bass.AP(tensor=cs_d, offset=0, ap=[[0, P], [1, NS]]),
            )

            xbf_s = x8_s = dx8_s = None
            if not do_xdma:
                if NBF:
                    xbf_s = consts.tile([P, NBF, MB], bf16, tag="xbfs")
                    nc.vector.memset(xbf_s, 0.5)
                x8_s = consts.tile([P, N8T, MB], f8e4, tag="x8s")
                nc.vector.memset(x8_s, 0.25)
                if NCORR:
                    dx8_s = consts.tile([P, NCORR, MB], f8e4, tag="dx8s")
                    nc.vector.memset(dx8_s, 0.25)

            for mb in range(NMB):
                m0 = mb * MB
                if do_xdma:
                    def load_x(name, src_d, n_tiles, dt, gran):
                        t = xpool.tile([P, n_tiles, MB], dt, tag=name,
                                       name=name)
                        k0 = 0
                        for H in gran:
                            src = src_d[k0 * P:(k0 + H) * P, m0:m0 + MB]
                            nc.sync.dma_start(
                                out=t[:, k0:k0 + H],
                                in_=src.rearrange("(b p) m -> p b m", p=P),
                            )
                            k0 += H
                        return t

                    first = (mb == 0)
                    if NBF:
                        xbf = load_x(
                            "xbf_t", x_d, NBF, bf16,
                            _granules(NBF, (1, 1, 2) if first else (), KB))
                        x8t = load_x("x8_t", x8_d, N8T, f8e4,
                                     _granules(N8T, (), 8))
                    else:
                        xbf = None
                        x8t = load_x(
                            "x8_t", x8_d, N8T, f8e4,
                            _granules(N8T, (2, 2, 4) if first else (), 8))
                    dx8t = (load_x("dx8_t", dx8_d, NCORR, f8e4,
                                   _granules(NCORR, (), 8))
                            if NCORR else None)
                else:
                    xbf, x8t, dx8t = xbf_s, x8_s, dx8_s
                if not do_mm:
                    continue

                ps = [psum_pool.tile([P, NS], f32, tag=f"ps{ms}",
                                     name=f"ps{ms}")
                      for ms in range(MSUB)]

                def emit(ms):
                    """Yield (kind, payload) product-instrs for bank ms in
                    k-load order (bf16 singles, then fp8 pairs)."""
                    msl = slice(ms * P, (ms + 1) * P)
                    ops = []
                    for kt in range(NBF):
                        ops.append((wd[:, kt], xbf[:, kt, msl], None))
                    for t in range(0, N8T, 2):
                        ops.append((w8t[:, t:t + 2, :],
                                    x8t[:, t:t + 2, msl], "dr"))
                    for c in range(0, NCORR, 2):
                        ops.append((w8t[:, c:c + 2, :],
                                    dx8t[:, c:c + 2, msl], "dr"))
                    for c in range(0, NCORR, 2):
                        ops.append((dw8t[:, c:c + 2, :],
                                    x8t[:, c:c + 2, msl], "dr"))
                    return ops

                def issue(ms, ops):
                    n = len(ops)
                    for i, (w_ap, x_ap, kind) in enumerate(ops):
                        nc.tensor.matmul(
                            ps[ms], x_ap, w_ap,
                            start=(i == 0), stop=(i == n - 1),
                            perf_mode=(mybir.MatmulPerfMode.DoubleRow
                                       if kind == "dr" else None),
                        )

                def interleaved(ops):
                    """bf16 and DoubleRow round-robin so DR LDWEIGHTS hides
                    under long bf16 matmuls."""
                    bf = [o for o in ops if o[2] is None]
                    dr = [o for o in ops if o[2] == "dr"]
                    out = []
                    while bf or dr:
                        if bf:
                            out.append(bf.pop(0))
                        if dr:
                            out.append(dr.pop(0))
                    return out

                if mb == 0:
                    # k-outer: consume x granules as they land, banks fill
                    # in parallel
                    per_bank = [emit(ms) for ms in range(MSUB)]
                    n = len(per_bank[0])
                    for i in range(n):
                        for ms in range(MSUB):
                            w_ap, x_ap, kind = per_bank[ms][i]
                            nc.tensor.matmul(
                                ps[ms], x_ap, w_ap,
                                start=(i == 0), stop=(i == n - 1),
                                perf_mode=(mybir.MatmulPerfMode.DoubleRow
                                           if kind == "dr" else None),
                            )
                else:
                    for ms in range(MSUB):
                        issue(ms, interleaved(emit(ms)))

                # evict: y = ps * colscale + bias, pairs -> one 1MB store on
                # the (otherwise idle) gpsimd queue.
                # Last block: per-bank granules to shrink the drain.
                def evict(ms, out_ap):
                    nc.vector.tensor_tensor(
                        out=out_ap, in0=ps[ms], in1=cs_t,
                        op=mybir.AluOpType.mult,
                    )
                    nc.vector.tensor_tensor(
                        out=out_ap, in0=out_ap, in1=bias_t,
                        op=mybir.AluOpType.add,
                    )

                if mb < NMB - 1:
                    for msp in range(MSUB // 2):
                        ot = opool.tile([P, 2, NS], f32, tag="ot")
                        for half in range(2):
                            evict(msp * 2 + half, ot[:, half])
                        row0 = m0 + msp * 2 * P
                        dst = y_d[row0:row0 + 2 * P, :]
                        nc.gpsimd.dma_start(
                            out=dst.rearrange("(b p) n -> p b n", p=P), in_=ot,
                        )
                else:
                    for ms in range(MSUB):
                        ot1 = opool.tile([P, 1, NS], f32, tag="ot1")
                        evict(ms, ot1[:, 0])
                        row0 = m0 + ms * P
                        dst = y_d[row0:row0 + P, :]
                        nc.gpsimd.dma_start(
                            out=dst.rearrange("(b p) n -> p b n", p=P), in_=ot1,
                        )

    nc.compile()
    return nc


def make_in_maps(x, scales, bias, weight_int8, col_indices, group_size):
    """Host-side sharding/layout prep: index gathers, dtype casts, and
    power-of-2 scale folding only."""
    e4 = ml_dtypes.float8_e4m3
    gs = int(group_size)
    x2 = np.asarray(x, dtype=np.float32).reshape(M, K)
    xT = np.ascontiguousarray(x2.T)                      # [K, M]

    ci = np.asarray(col_indices).astype(np.int64)
    inv = np.argsort(ci)                     # inv[j]: W row paired with x col j
    gi = inv // gs                           # scale group per permuted row

    Wp = np.asarray(weight_int8)[inv].astype(np.float32)   # [K, N]
    sc = np.asarray(scales, dtype=np.float32)[gi]          # [K, N] expanded
    wdq = Wp * sc                                          # [K, N] f32
    bias = np.asarray(bias, dtype=np.float32)

    # per-column power-of-2 normalizer: max|wd_n| * 2^cn in (120, 240]
    mxc = np.abs(wdq).max(axis=0)
    cn = np.floor(np.log2(240.0 / np.maximum(mxc, 1e-30))).astype(np.float32)
    cn = np.minimum(cn, 30.0)
    colscale = (2.0 ** -(A8 + cn)).astype(np.float32)

    kb = slice(0, NBF * P)
    k8 = slice(NBF * P, K)
    kc = slice(NBF * P, (NBF + NCORR) * P)

    full = {}
    if NBF:
        full["xbf"] = xT[kb].astype(ml_dtypes.bfloat16)
        full["wbf"] = (wdq[kb] * 2.0 ** (A8 + cn)).astype(ml_dtypes.bfloat16)
    xs = np.clip(xT[k8] * float(2 ** A8), -240, 240)
    x8 = xs.astype(e4)
    full["x8"] = x8
    ws = wdq[k8] * 2.0 ** cn                       # |ws| <= 240 by cn
    w8 = ws.astype(e4)
    full["w8"] = w8
    if NCORR:
        nrows = NCORR * P
        full["dx8"] = (xs[:nrows] - x8[:nrows].astype(np.float32)).astype(e4)
        full["dw8"] = (ws[:nrows] - w8[:nrows].astype(np.float32)).astype(e4)

    in_maps = []
    for c in range(NCORES):
        cols = slice(c * NS, (c + 1) * NS)
        m = {k: full[k] for k in ("xbf", "x8", "dx8") if k in full}
        for k in ("wbf", "w8", "dw8"):
            if k in full:
                m[k] = np.ascontiguousarray(full[k][:, cols])
        m["bias"] = bias[cols]
        m["colscale"] = colscale[cols]
        in_maps.append(m)
    return in_maps


_RUNNER = None

_REPL = ("xbf", "x8", "dx8")        # tensors identical on every core


def _make_runner():
    """Build the bass module once and wrap it in a cached sharded jit."""
    import jax
    from jax.sharding import Mesh, PartitionSpec, NamedSharding
    from jax.experimental.shard_map import shard_map
    from concourse import bass2jax
    from concourse.bass2jax import _bass_exec_p, install_neuronx_cc_hook

    nc = build(repeats=1)
    install_neuronx_cc_hook()
    partition_name = nc.partition_id_tensor.name if nc.partition_id_tensor else None

    in_names, out_names, out_avals, zero_outs = [], [], [], []
    for alloc in nc.m.functions[0].allocations:
        if not isinstance(alloc, mybir.MemoryLocationSet):
            continue
        name = alloc.memorylocations[0].name
        if alloc.kind == "ExternalInput":
            if name != partition_name:
                in_names.append(name)
        elif alloc.kind == "ExternalOutput":
            out_names.append(name)
            shape = tuple(alloc.tensor_shape)
            dtype = mybir.dt.np(alloc.dtype)
            out_avals.append(jax.core.ShapedArray(shape, dtype))
            zero_outs.append(np.zeros(shape, dtype))
    all_in_names = list(in_names) + list(out_names)
    if partition_name is not None:
        all_in_names.append(partition_name)
    n_params, n_outs = len(in_names), len(out_names)

    def _body(*args):
        operands = list(args)
        if partition_name is not None:
            operands.append(bass2jax.partition_id_tensor())
        outs = _bass_exec_p.bind(
            *operands,
            out_avals=tuple(out_avals),
            in_names=tuple(all_in_names),
            out_names=tuple(out_names),
            lowering_input_output_aliases=(),
            sim_require_finite=True,
            sim_require_nnan=True,
            nc=nc,
        )
        return tuple(outs)

    devices = jax.devices()[:NCORES]
    mesh = Mesh(np.asarray(devices), ("core",))
    # x tensors are identical on every core: pass them replicated so only one
    # copy crosses the host->device link; per-core tensors are concat-sharded.
    in_specs = tuple(
        PartitionSpec() if name in _REPL else PartitionSpec("core")
        for name in in_names
    ) + (PartitionSpec("core"),) * n_outs
    sharded = jax.jit(
        shard_map(
            _body, mesh=mesh,
            in_specs=in_specs,
            out_specs=(PartitionSpec("core"),) * n_outs,
            check_rep=False,
        ),
        keep_unused=True,
    )
    shard_core = NamedSharding(mesh, PartitionSpec("core"))
    shard_repl = NamedSharding(mesh, PartitionSpec())

    def run(in_maps):
        import jax as _jax
        dev_in = []
        for name in in_names:
            if name in _REPL:
                dev_in.append(
                    _jax.device_put(np.asarray(in_maps[0][name]), shard_repl))
            else:
                a = np.concatenate(
                    [np.asarray(in_maps[c][name]) for c in range(NCORES)], axis=0)
                dev_in.append(_jax.device_put(a, shard_core))
        dev_zero = [
            _jax.device_put(
                np.zeros((NCORES * z.shape[0], *z.shape[1:]), z.dtype), shard_core)
            for z in zero_outs
        ]
        out = sharded(*dev_in, *dev_zero)
        return [
            {name: np.asarray(out[i]).reshape(NCORES, *zero_outs[i].shape)[c]
             for i, name in enumerate(out_names)}
            for c in range(NCORES)
        ]

    return run


def kernel(x, scales, bias, weight_int8, col_indices, group_size):
    global _RUNNER
    in_maps = make_in_maps(x, scales, bias, weight_int8, col_indices, group_size)
    if _RUNNER is None:
        _RUNNER = _make_runner()
    results = _RUNNER(in_maps)
    y = np.concatenate([results[c]["y"] for c in range(NCORES)], axis=1)
    return np.ascontiguousarray(y.reshape(B, S, N))


# revision 15
# speedup vs baseline: 1.2607x; 1.2607x over previous
"""CPR linear (int8-dequant matmul with column reordering) on 8 Trainium2
NeuronCores.

Math: y = x[:, col_indices] @ (W_int8 * repeat(scales, gs)) + bias
Equivalently, with inv = argsort(col_indices):
    y[m, n] = sum_j x[m, j] * W[inv[j], n] * scales[inv[j]//gs, n]
so x is consumed in natural column order and the permutation rides on W's
rows (host-side index gather; W is 8x smaller than x).

Sharding: column-parallel. Each core owns 512 output features: its slice
of W (row-permuted, dequantized on host) and bias; x is replicated.

Precision/speed scheme along K (32 k-tiles of 128), per-tile class:
  - bf16:  plain bf16 matmul (213ns/tile)
  - corr:  fp8e4m3 with first-order residual correction:
               x8@w8 + dx8@w8 + x8@dw8
           (3 fp8 products = 1.5 DoubleRow instrs; ~bf16 accuracy since the
           residuals kill both operands' quantization noise; only the tiny
           dx*dw cross term is dropped)
  - plain: fp8e4m3 pair (0.5 DoubleRow instr; ~3.75% tile noise)
DoubleRow contracts 2 k-tiles per PE pass at 0.5 cycles/row. The host folds
a per-column power-of-2 scale 2^(A8+cn) into every weight format so all
products accumulate in one PSUM group; eviction rescales by 2^-(A8+cn)
and adds bias. fp8 residuals use the SAME power-of-2 scales so everything
lands at matched magnitude.

Per-core device kernel:
  - weights DMA'd into resident tiles, chunked so the first matmuls gate on
    a ~0.25MB load; bias/colscale broadcast [512] -> [128, 512]
  - loop over 8 m-blocks of 1024 rows:
      x loads on the sync queue ([128, t, 1024] tiles per dtype)
      mb 0: k-outer over all 8 PSUM banks so the PE starts on the first
            x granule instead of waiting for the full block
      mb 1+: m-subtile-outer, bf16/DoubleRow interleaved so DoubleRow
            LDWEIGHTS can hide under long bf16 matmuls
      PSUM * colscale + bias -> SBUF pairs -> stores on the gpsimd queue
"""
from contextlib import ExitStack

import numpy as np
import ml_dtypes

import concourse.bass as bass
import concourse.bacc as bacc
import concourse.mybir as mybir
import concourse.tile as tile

B, S, K, N = 4, 2048, 4096, 4096
M = B * S                    # 8192
NCORES = 8
NS = N // NCORES             # 512 output cols per core
P = 128
NKT = K // P                 # 32 k-tiles
MB = 1024                    # m-block rows
NMB = M // MB                # 8
MSUB = MB // P               # 8

# k-tile classes: [NBF bf16][NCORR fp8-corrected][NPLAIN fp8-plain]
NBF, NCORR, NPLAIN = 18, 8, 6
N8T = NCORR + NPLAIN         # tiles with fp8 x8/w8 operands
assert NBF + N8T == NKT and N8T % 2 == 0 and NCORR % 2 == 0
A8 = 5                       # x fp8 pre-scale: x*2^5 (max |x|*32 < 240)

bf16 = mybir.dt.bfloat16
f32 = mybir.dt.float32
f8e4 = mybir.dt.float8e4

KB = 4                       # k-tiles per steady-state x DMA


def _granules(n_tiles, first_fine, fine):
    """Chunk n_tiles into DMA granule sizes, finest first."""
    out, left = [], n_tiles
    sched = list(first_fine) if first_fine else []
    for g in sched:
        if left <= 0:
            break
        g = min(g, left)
        out.append(g)
        left -= g
    while left > 0:
        g = min(fine, left)
        out.append(g)
        left -= g
    return out


def build(repeats: int = 1, variant: str = "full"):
    """variant: "full" | "nomm" (DMA path only) | "mmonly" (PE path only)"""
    do_mm = variant in ("full", "mmonly")
    do_xdma = variant in ("full", "nomm")

    nc = bacc.Bacc(None)
    # x pre-transposed on host, split by k-tile class
    x_d = x8_d = dx8_d = None
    w_d = w8_d = dw8_d = None
    if NBF:
        x_d = nc.dram_tensor("xbf", [NBF * P, M], bf16, kind="ExternalInput")
        w_d = nc.dram_tensor("wbf", [NBF * P, NS], bf16, kind="ExternalInput")
    x8_d = nc.dram_tensor("x8", [N8T * P, M], f8e4, kind="ExternalInput")
    w8_d = nc.dram_tensor("w8", [N8T * P, NS], f8e4, kind="ExternalInput")
    if NCORR:
        dx8_d = nc.dram_tensor("dx8", [NCORR * P, M], f8e4,
                               kind="ExternalInput")
        dw8_d = nc.dram_tensor("dw8", [NCORR * P, NS], f8e4,
                               kind="ExternalInput")
    b_d = nc.dram_tensor("bias", [NS], f32, kind="ExternalInput")
    cs_d = nc.dram_tensor("colscale", [NS], f32, kind="ExternalInput")
    y_d = nc.dram_tensor("y", [M, NS], f32, kind="ExternalOutput")

    with tile.TileContext(nc) as tc, ExitStack() as stk:
        if repeats > 1:
            stk.enter_context(tc.For_i(0, repeats, 1))
        with (
            tc.tile_pool(name="consts", bufs=1) as consts,
            tc.tile_pool(name="xpool", bufs=2) as xpool,
            tc.tile_pool(name="opool", bufs=2) as opool,
            tc.tile_pool(name="psum", bufs=1, space="PSUM") as psum_pool,
        ):
            # merged per-bank op order: DoubleRow LDWEIGHTS only hides under
            # a preceding bf16 matmul, so spread DRs evenly among bf16 ops
            bf_ops = [("bf", kt) for kt in range(NBF)]
            dr_ops = ([("x8", t) for t in range(0, N8T, 2)]
                      + [("dx", c) for c in range(0, NCORR, 2)]
                      + [("dw", c) for c in range(0, NCORR, 2)])
            nb, nd = len(bf_ops), len(dr_ops)
            merged, ib, idr = [], 0, 0
            while ib < nb or idr < nd:
                if idr >= nd or (ib < nb
                                 and (ib + 1) * nd <= (idr + 1) * nb):
                    merged.append(bf_ops[ib]); ib += 1
                else:
                    merged.append(dr_ops[idr]); idr += 1
            n_ops = len(merged)

            # mb0 x-granule schedules (fine so the PE starts early) and
            # their first-use order under `merged`
            gran0 = {"xbf": _granules(NBF, (1, 1, 2), KB) if NBF else [],
                     "x8": _granules(N8T, (), 2),
                     "dx8": _granules(NCORR, (), 2) if NCORR else []}

            def granule_idx(gran, tile_i):
                k0 = 0
                for gi, H in enumerate(gran):
                    if tile_i < k0 + H:
                        return gi
                    k0 += H
                raise AssertionError

            first_use = []      # [(tensor, granule_idx)] in merged order
            seen = set()
            for kind, t in merged:
                needs = ([("xbf", granule_idx(gran0["xbf"], t))]
                         if kind == "bf" else
                         [("dx8", granule_idx(gran0["dx8"], t)),
                          ("dx8", granule_idx(gran0["dx8"], t + 1))]
                         if kind == "dx" else
                         [("x8", granule_idx(gran0["x8"], t)),
                          ("x8", granule_idx(gran0["x8"], t + 1))])
                for need in needs:
                    if need not in seen:
                        seen.add(need)
                        first_use.append(need)
            # any unused granules (shouldn't happen) appended for safety
            for name in ("xbf", "x8", "dx8"):
                for gi in range(len(gran0[name])):
                    if (name, gi) not in seen:
                        first_use.append((name, gi))

            # resident weights, chunked smallest-first so early matmuls
            # gate on a small load
            def load_w(dst, src, gran):
                k0 = 0
                for H in gran:
                    nc.scalar.dma_start(
                        out=dst[:, k0:k0 + H],
                        in_=src[k0 * P:(k0 + H) * P, :].rearrange(
                            "(t p) n -> p t n", p=P))
                    k0 += H

            wd = (consts.tile([P, NBF, NS], bf16, name="wd")
                  if NBF else None)
            w8t = consts.tile([P, N8T, NS], f8e4, name="w8t")
            dw8t = (consts.tile([P, NCORR, NS], f8e4, name="dw8t")
                    if NCORR else None)

            # weight granules in merged-op first-use order, so the mb0
            # k-outer stream never waits on the weight queue
            wgran = {"xbf": _granules(NBF, (2, 2), 4) if NBF else [],
                     "x8": _granules(N8T, (), 2),
                     "dx8": _granules(NCORR, (), 2) if NCORR else []}
            wtens = {"xbf": (wd, w_d), "x8": (w8t, w8_d),
                     "dx8": (dw8t, dw8_d)}
            wfirst = []
            wseen = set()
            for kind, t in merged:
                needs = ([("xbf", granule_idx(wgran["xbf"], t))]
                         if kind == "bf" else
                         [("dx8", granule_idx(wgran["dx8"], t)),
                          ("dx8", granule_idx(wgran["dx8"], t + 1))]
                         if kind == "dw" else
                         [("x8", granule_idx(wgran["x8"], t)),
                          ("x8", granule_idx(wgran["x8"], t + 1))])
                for need in needs:
                    if need not in wseen:
                        wseen.add(need)
                        wfirst.append(need)
            for name in ("xbf", "x8", "dx8"):
                for gi in range(len(wgran[name])):
                    if (name, gi) not in wseen:
                        wfirst.append((name, gi))
            for name, gi in wfirst:
                dst, src = wtens[name]
                k0 = sum(wgran[name][:gi])
                H = wgran[name][gi]
                nc.scalar.dma_start(
                    out=dst[:, k0:k0 + H],
                    in_=src[k0 * P:(k0 + H) * P, :].rearrange(
                        "(t p) n -> p t n", p=P))

            # bias/colscale broadcast to all partitions (needed only at first
            # PSUM eviction, so issued after the W loads on the same queue)
            bias_t = consts.tile([P, NS], f32)
            nc.scalar.dma_start(
                out=bias_t,
                in_=bass.AP(tensor=b_d, offset=0, ap=[[0, P], [1, NS]]),
            )
            cs_t = consts.tile([P, NS], f32)
            nc.scalar.dma_start(
                out=cs_t,
                in_=bass.AP(tensor=cs_d, offset=0, ap=[[0, P], [1, NS]]),
            )

            xbf_s = x8_s = dx8_s = None
            if not do_xdma:
                if NBF:
                    xbf_s = consts.tile([P, NBF, MB], bf16, tag="xbfs")
                    nc.vector.memset(xbf_s, 0.5)
                x8_s = consts.tile([P, N8T, MB], f8e4, tag="x8s")
                nc.vector.memset(x8_s, 0.25)
                if NCORR:
                    dx8_s = consts.tile([P, NCORR, MB], f8e4, tag="dx8s")
                    nc.vector.memset(dx8_s, 0.25)

            for mb in range(NMB):
                m0 = mb * MB
                if do_xdma:
                    def alloc_x(name, n_tiles, dt):
                        return xpool.tile([P, n_tiles, MB], dt, tag=name,
                                          name=name)

                    def granule_dma(t, src_d, gran, gi):
                        k0 = sum(gran[:gi])
                        H = gran[gi]
                        src = src_d[k0 * P:(k0 + H) * P, m0:m0 + MB]
                        nc.sync.dma_start(
                            out=t[:, k0:k0 + H],
                            in_=src.rearrange("(b p) m -> p b m", p=P),
                        )

                    xbf = alloc_x("xbf_t", NBF, bf16) if NBF else None
                    x8t = alloc_x("x8_t", N8T, f8e4)
                    dx8t = alloc_x("dx8_t", NCORR, f8e4) if NCORR else None
                    tens = {"xbf": (xbf, x_d), "x8": (x8t, x8_d),
                            "dx8": (dx8t, dx8_d)}
                    if mb == 0:
                        # granules in first-use order of the merged op list
                        for name, gi in first_use:
                            t, src_d = tens[name]
                            granule_dma(t, src_d, gran0[name], gi)
                    else:
                        for name in ("xbf", "x8", "dx8"):
                            t, src_d = tens[name]
                            if t is None:
                                continue
                            n_tiles = {"xbf": NBF, "x8": N8T,
                                       "dx8": NCORR}[name]
                            gran = _granules(n_tiles, (),
                                             KB if name == "xbf" else 8)
                            for gi in range(len(gran)):
                                granule_dma(t, src_d, gran, gi)
                else:
                    xbf, x8t, dx8t = xbf_s, x8_s, dx8_s
                if not do_mm:
                    continue

                ps = [psum_pool.tile([P, NS], f32, tag=f"ps{ms}",
                                     name=f"ps{ms}")
                      for ms in range(MSUB)]

                def issue_op(op, ms, i):
                    kind, t = op
                    msl = slice(ms * P, (ms + 1) * P)
                    if kind == "bf":
                        x_ap, w_ap, pm = xbf[:, t, msl], wd[:, t], None
                    elif kind == "x8":
                        x_ap, w_ap = x8t[:, t:t + 2, msl], w8t[:, t:t + 2, :]
                        pm = mybir.MatmulPerfMode.DoubleRow
                    elif kind == "dx":
                        x_ap, w_ap = dx8t[:, t:t + 2, msl], w8t[:, t:t + 2, :]
                        pm = mybir.MatmulPerfMode.DoubleRow
                    else:  # "dw"
                        x_ap, w_ap = x8t[:, t:t + 2, msl], dw8t[:, t:t + 2, :]
                        pm = mybir.MatmulPerfMode.DoubleRow
                    nc.tensor.matmul(
                        ps[ms], x_ap, w_ap,
                        start=(i == 0), stop=(i == n_ops - 1),
                        perf_mode=pm,
                    )

                if mb == 0:
                    # k-outer across all banks, two merged ops per flush so
                    # bf16/DR still alternate at instruction granularity
                    for i0 in range(0, n_ops, 2):
                        chunk = merged[i0:i0 + 2]
                        for ms in range(MSUB):
                            for j, op in enumerate(chunk):
                                issue_op(op, ms, i0 + j)
                else:
                    for ms in range(MSUB):
                        for i, op in enumerate(merged):
                            issue_op(op, ms, i)

                # evict: y = ps * colscale + bias, pairs -> one 1MB store on
                # the (otherwise idle) gpsimd queue.
                # Last block: per-bank granules to shrink the drain.
                def evict(ms, out_ap):
                    nc.vector.tensor_tensor(
                        out=out_ap, in0=ps[ms], in1=cs_t,
                        op=mybir.AluOpType.mult,
                    )
                    nc.vector.tensor_tensor(
                        out=out_ap, in0=out_ap, in1=bias_t,
                        op=mybir.AluOpType.add,
                    )

                if mb < NMB - 1:
                    for msp in range(MSUB // 2):
                        ot = opool.tile([P, 2, NS], f32, tag="ot")
                        for half in range(2):
                            evict(msp * 2 + half, ot[:, half])
                        row0 = m0 + msp * 2 * P
                        dst = y_d[row0:row0 + 2 * P, :]
                        nc.gpsimd.dma_start(
                            out=dst.rearrange("(b p) n -> p b n", p=P), in_=ot,
                        )
                else:
                    for ms in range(MSUB):
                        ot1 = opool.tile([P, 1, NS], f32, tag="ot1")
                        evict(ms, ot1[:, 0])
                        row0 = m0 + ms * P
                        dst = y_d[row0:row0 + P, :]
                        nc.gpsimd.dma_start(
                            out=dst.rearrange("(b p) n -> p b n", p=P), in_=ot1,
                        )

    nc.compile()
    return nc


def make_in_maps(x, scales, bias, weight_int8, col_indices, group_size):
    """Host-side sharding/layout prep: index gathers, dtype casts, and
    power-of-2 scale folding only."""
    e4 = ml_dtypes.float8_e4m3
    gs = int(group_size)
    x2 = np.asarray(x, dtype=np.float32).reshape(M, K)
    xT = np.ascontiguousarray(x2.T)                      # [K, M]

    ci = np.asarray(col_indices).astype(np.int64)
    inv = np.argsort(ci)                     # inv[j]: W row paired with x col j
    gi = inv // gs                           # scale group per permuted row

    Wp = np.asarray(weight_int8)[inv].astype(np.float32)   # [K, N]
    sc = np.asarray(scales, dtype=np.float32)[gi]          # [K, N] expanded
    wdq = Wp * sc                                          # [K, N] f32
    bias = np.asarray(bias, dtype=np.float32)

    # per-column power-of-2 normalizer: max|wd_n| * 2^cn in (120, 240]
    mxc = np.abs(wdq).max(axis=0)
    cn = np.floor(np.log2(240.0 / np.maximum(mxc, 1e-30))).astype(np.float32)
    cn = np.minimum(cn, 30.0)
    colscale = (2.0 ** -(A8 + cn)).astype(np.float32)

    kb = slice(0, NBF * P)
    k8 = slice(NBF * P, K)
    kc = slice(NBF * P, (NBF + NCORR) * P)

    full = {}
    if NBF:
        full["xbf"] = xT[kb].astype(ml_dtypes.bfloat16)
        full["wbf"] = (wdq[kb] * 2.0 ** (A8 + cn)).astype(ml_dtypes.bfloat16)
    xs = np.clip(xT[k8] * float(2 ** A8), -240, 240)
    x8 = xs.astype(e4)
    full["x8"] = x8
    ws = wdq[k8] * 2.0 ** cn                       # |ws| <= 240 by cn
    w8 = ws.astype(e4)
    full["w8"] = w8
    if NCORR:
        nrows = NCORR * P
        full["dx8"] = (xs[:nrows] - x8[:nrows].astype(np.float32)).astype(e4)
        full["dw8"] = (ws[:nrows] - w8[:nrows].astype(np.float32)).astype(e4)

    in_maps = []
    for c in range(NCORES):
        cols = slice(c * NS, (c + 1) * NS)
        m = {k: full[k] for k in ("xbf", "x8", "dx8") if k in full}
        for k in ("wbf", "w8", "dw8"):
            if k in full:
                m[k] = np.ascontiguousarray(full[k][:, cols])
        m["bias"] = bias[cols]
        m["colscale"] = colscale[cols]
        in_maps.append(m)
    return in_maps


_RUNNER = None

_REPL = ("xbf", "x8", "dx8")        # tensors identical on every core


def _make_runner():
    """Build the bass module once and wrap it in a cached sharded jit."""
    import jax
    from jax.sharding import Mesh, PartitionSpec, NamedSharding
    from jax.experimental.shard_map import shard_map
    from concourse import bass2jax
    from concourse.bass2jax import _bass_exec_p, install_neuronx_cc_hook

    nc = build(repeats=1)
    install_neuronx_cc_hook()
    partition_name = nc.partition_id_tensor.name if nc.partition_id_tensor else None

    in_names, out_names, out_avals, zero_outs = [], [], [], []
    for alloc in nc.m.functions[0].allocations:
        if not isinstance(alloc, mybir.MemoryLocationSet):
            continue
        name = alloc.memorylocations[0].name
        if alloc.kind == "ExternalInput":
            if name != partition_name:
                in_names.append(name)
        elif alloc.kind == "ExternalOutput":
            out_names.append(name)
            shape = tuple(alloc.tensor_shape)
            dtype = mybir.dt.np(alloc.dtype)
            out_avals.append(jax.core.ShapedArray(shape, dtype))
            zero_outs.append(np.zeros(shape, dtype))
    all_in_names = list(in_names) + list(out_names)
    if partition_name is not None:
        all_in_names.append(partition_name)
    n_params, n_outs = len(in_names), len(out_names)

    def _body(*args):
        operands = list(args)
        if partition_name is not None:
            operands.append(bass2jax.partition_id_tensor())
        outs = _bass_exec_p.bind(
            *operands,
            out_avals=tuple(out_avals),
            in_names=tuple(all_in_names),
            out_names=tuple(out_names),
            lowering_input_output_aliases=(),
            sim_require_finite=True,
            sim_require_nnan=True,
            nc=nc,
        )
        return tuple(outs)

    devices = jax.devices()[:NCORES]
    mesh = Mesh(np.asarray(devices), ("core",))
    # x tensors are identical on every core: pass them replicated so only one
    # copy crosses the host->device link; per-core tensors are concat-sharded.
    in_specs = tuple(
        PartitionSpec() if name in _REPL else PartitionSpec("core")
        for name in in_names
    ) + (PartitionSpec("core"),) * n_outs
    sharded = jax.jit(
        shard_map(
            _body, mesh=mesh,
            in_specs=in_specs,
            out_specs=(PartitionSpec("core"),) * n_outs,
            check_rep=False,
        ),
        keep_unused=True,
    )
    shard_core = NamedSharding(mesh, PartitionSpec("core"))
    shard_repl = NamedSharding(mesh, PartitionSpec())

    def run(in_maps):
        import jax as _jax
        dev_in = []
        for name in in_names:
            if name in _REPL:
                dev_in.append(
                    _jax.device_put(np.asarray(in_maps[0][name]), shard_repl))
            else:
                a = np.concatenate(
                    [np.asarray(in_maps[c][name]) for c in range(NCORES)], axis=0)
                dev_in.append(_jax.device_put(a, shard_core))
        dev_zero = [
            _jax.device_put(
                np.zeros((NCORES * z.shape[0], *z.shape[1:]), z.dtype), shard_core)
            for z in zero_outs
        ]
        out = sharded(*dev_in, *dev_zero)
        return [
            {name: np.asarray(out[i]).reshape(NCORES, *zero_outs[i].shape)[c]
             for i, name in enumerate(out_names)}
            for c in range(NCORES)
        ]

    return run


def kernel(x, scales, bias, weight_int8, col_indices, group_size):
    global _RUNNER
    in_maps = make_in_maps(x, scales, bias, weight_int8, col_indices, group_size)
    if _RUNNER is None:
        _RUNNER = _make_runner()
    results = _RUNNER(in_maps)
    y = np.concatenate([results[c]["y"] for c in range(NCORES)], axis=1)
    return np.ascontiguousarray(y.reshape(B, S, N))
